# revision 20
# baseline (speedup 1.0000x reference)
"""Trainium2 Bass kernel for nn_MlpwithSOMModule (sum-of-max hard-attention score).

Math identity used: the reference computes
    sim  = ctx @ ent^T            # [L, M] per (b, k)
    idx  = argmax_m sim
    out  = sum_l dot(ctx_l, ent_idx[l]) = sum_l sim[l, idx[l]] = sum_l max_m sim[l, m]
so no gather/argmax is needed on device - just matmul, row-max, and a sum.

Sharding: B = 8 == n_cores, so core c processes context[c] = [64, 2, 256, 768]
(its 64 (b,k) pairs are exactly batch b == c). No cross-core communication.

Per-core pipeline (Tile framework):
  1. SWDGE DMA loads ONE pair per dma_start (63 loads + a 3-piece final pair),
     casting fp32 -> bf16 in the DMA. Tokens map to partitions as l = 2p + r
     so each partition reads 6KB contiguous bursts. Per-pair loads give the
     finest possible semaphore granularity: the PE never waits on more than
     one pair's worth of HBM bytes, so its lag behind the DMA stream stays
     bounded at ~1 pair instead of growing to a multi-pair backlog that gets
     exposed as a tail after the last byte lands. xpool is 24 buffers deep so
     dma_start issue runs far ahead of PE consumption and the SDMA queue
     never starves (issue is gated by buffer recycling; shallow buffering
     couples DMA issue to PE progress and collapses the sustained HBM rate
     from ~425 GB/s to ~330).
  2. PE transposes 24 [128,128] bf16 blocks per pair (ctx and ent to
     [d, token]) into bf16 PSUM slabs; ACT/DVE copy 1024-wide slabs to SBUF.
     The PE stream staggers one pair: [T(p+1), MM(p)] so MM(p) never waits
     on its own slab copies.
  3. 12 accumulating bf16 matmuls per pair: S[l, m] (contraction d = 6 x 128);
     DVE reduce_max over m -> row maxes in a [128, 128] SBUF tile (RM).
  4. Final sum over l = ones^T @ RM, split in two halves: the first half is
     emitted right after pair 31 so only the second 64-column matmul plus a
     copy/fold/store chain depends on the last pair.
"""

import sys

for _p in ("/opt/trn_rl_repo", "/root/.axon_site/_ro/trn_rl_repo"):
    if _p not in sys.path:
        sys.path.insert(0, _p)

import numpy as np

B, TOPK, L, D = 8, 64, 256, 768
N_CORES = 8
PAIRS_PER_CORE = 64  # == TOPK; one batch index per core
P = 128
DCHUNKS = D // P  # 6
LCHUNKS = L // P  # 2

_cache = {}


def _build():
    import concourse.bass as bass
    import concourse.mybir as mybir
    from concourse import bacc
    from concourse.tile import TileContext
    from concourse.masks import make_identity

    nc = bacc.Bacc(
        "TRN2",
        target_bir_lowering=False,
        debug=False,
        num_devices=N_CORES,
    )

    x = nc.dram_tensor(
        "x", [PAIRS_PER_CORE, 2, L, D], mybir.dt.float32, kind="ExternalInput"
    ).ap()
    out = nc.dram_tensor(
        "out", [1, PAIRS_PER_CORE], mybir.dt.float32, kind="ExternalOutput"
    ).ap()

    bf16 = mybir.dt.bfloat16
    f32 = mybir.dt.float32

    # DRAM view: pair pr, partition p, chunk c=(s,r), d -- token l = 2p + r,
    # so each partition reads 2 consecutive rows (6KB contiguous DMA bursts).
    # The token permutation is harmless: out = sum_l max_m is invariant to the
    # order of l and m, and the permutation is identical across all d-chunks.
    xv = x.rearrange("pr s (p two) d -> pr p s two d", p=P, two=2)

    with TileContext(nc) as tc:
        with (
            tc.tile_pool(name="const", bufs=1) as cpool,
            tc.tile_pool(name="xload", bufs=24) as xpool,
            tc.tile_pool(name="xfinal", bufs=3) as xfpool,
            tc.tile_pool(name="tpose", bufs=3) as tpool,
            tc.tile_pool(name="ppose", bufs=4, space="PSUM") as ppool,
            tc.tile_pool(name="pmm", bufs=3, space="PSUM") as mpool,
            tc.tile_pool(name="pfin", bufs=1, space="PSUM") as fpool,
        ):
            ident = cpool.tile([P, P], bf16)
            ones = cpool.tile([P, 1], f32)
            # row maxes: column 2*pair+lc holds max_m S[l, m] for l-chunk lc
            RM = cpool.tile([P, 2 * PAIRS_PER_CORE], f32)
            fin = fpool.tile([1, 2 * PAIRS_PER_CORE], f32)

            def emit_mm(pair, T):
                ps = mpool.tile([P, LCHUNKS, 2 * P], f32)
                for lc in range(LCHUNKS):
                    for dc in range(DCHUNKS):
                        off = (dc * 2 + lc) * P
                        nc.tensor.matmul(
                            ps[:, lc],
                            T[:, off : off + P],  # ctxT block [d, l-chunk]
                            T[:, 1536 + dc * 2 * P : 1536 + (dc + 1) * 2 * P],
                            start=(dc == 0),
                            stop=(dc == DCHUNKS - 1),
                        )
                nc.vector.reduce_max(
                    RM[:, 2 * pair : 2 * pair + 2], ps, axis=mybir.AxisListType.X
                )
                if pair == PAIRS_PER_CORE // 2 - 1:
                    # first half of the final sum over l: only the second half
                    # remains on the critical path after the last pair
                    nc.tensor.matmul(
                        fin[:, : PAIRS_PER_CORE],
                        ones,
                        RM[:, : PAIRS_PER_CORE],
                        start=True,
                        stop=True,
                    )

            # Load plan: one pair per dma_start. Finest semaphore
            # granularity (PE backlog stays ~1 pair when the stream ends) and
            # the smoothest HBM demand curve; with the 24-deep xpool the
            # descgen issue runs far ahead of PE consumption. The final pair
            # is loaded in 3 pieces below.
            sizes = [1] * (PAIRS_PER_CORE - 1)
            assert sum(sizes) == PAIRS_PER_CORE - 1
            loads = []
            s0 = 0
            for n in sizes:
                loads.append((s0, n))
                s0 += n

            prev = None
            for li, (p0, n) in enumerate(loads):
                Xfull = xpool.tile([P, 4, D], bf16, tag="X", name="X")
                X = Xfull[:, : 4 * n, :]
                # fp32 -> bf16 cast in DMA (SWDGE)
                nc.gpsimd.dma_start(
                    X, xv[p0 : p0 + n].rearrange("n p s two d -> p (n s) (two d)")
                )
                if li == 0:
                    # emitted after the first dma_start so the Q7/SWDGE engine
                    # starts descriptor generation immediately at kernel start
                    make_identity(nc, ident)
                    nc.gpsimd.memset(ones, 1.0)
                for q in range(n):
                    if prev is not None:
                        emit_mm(*prev)
                    # T: ctxT at [0, 1536), entT at [1536, 3072); block
                    # (t, dc, lc) lives at free offset 128*(t*12 + dc*2 + lc)
                    T = tpool.tile([P, 2 * 1536], bf16)
                    for jj in range(3):
                        psb = ppool.tile([P, 1024], bf16)
                        for slot in range(8):
                            j = jj * 8 + slot
                            t, rem = divmod(j, 12)
                            dc, lc = divmod(rem, 2)
                            c = q * 4 + t * 2 + lc
                            nc.tensor.transpose(
                                psb[:, slot * P : (slot + 1) * P],
                                X[:, c, dc * P : (dc + 1) * P],
                                ident,
                            )
                        # deterministic 2:1 ACT/DVE split keeps DVE light (it
                        # also runs the per-pair reduce_max) and ACT under the
                        # per-pair load cadence
                        dst = T[:, jj * 1024 : (jj + 1) * 1024]
                        if jj == 1:
                            nc.vector.tensor_copy(dst, psb)
                        else:
                            nc.scalar.copy(dst, psb)
                    prev = (p0 + q, T)

            # Final pair (63): loaded in 3 pieces (ent, ctx l-chunk 0, ctx
            # l-chunk 1) so the only compute gated on the very last HBM bytes
            # is 6 transposes + 6 matmuls instead of a full pair.
            LAST = PAIRS_PER_CORE - 1
            xl = xv[LAST]
            XE = xfpool.tile([P, 2, D], bf16, name="XE")
            nc.gpsimd.dma_start(XE, xl[:, 1])
            XC0 = xfpool.tile([P, 1, D], bf16, name="XC0")
            nc.gpsimd.dma_start(XC0, xl[:, 0, 0:1, :])
            XC1 = xfpool.tile([P, 1, D], bf16, name="XC1")
            nc.gpsimd.dma_start(XC1, xl[:, 0, 1:2, :])

            emit_mm(*prev)

            T = tpool.tile([P, 2 * 1536], bf16, tag="T", name="Tlast")
            Tc = T[:, :1536].rearrange("p (dc two f) -> p dc two f", two=2, f=P)
            # ent blocks j=12..23 -> T[1536:3072]
            psbA = ppool.tile([P, 1024], bf16, tag="psb", name="psbA")
            for k in range(8):
                dc, lc = divmod(k, 2)
                nc.tensor.transpose(
                    psbA[:, k * P : (k + 1) * P], XE[:, lc, dc * P : (dc + 1) * P], ident
                )
            nc.any.tensor_copy(T[:, 1536:2560], psbA)
            psbB = ppool.tile([P, 1024], bf16, tag="psb", name="psbB")
            for k in range(4):
                dc, lc = divmod(k + 8, 2)
                nc.tensor.transpose(
                    psbB[:, k * P : (k + 1) * P], XE[:, lc, dc * P : (dc + 1) * P], ident
                )
            nc.any.tensor_copy(T[:, 2560:3072], psbB[:, :512])
            ps = mpool.tile([P, LCHUNKS, 2 * P], f32, tag="ps", name="ps_last")
            for lc, XC in ((0, XC0), (1, XC1)):
                psbC = ppool.tile([P, 1024], bf16, tag="psb", name=f"psbC{lc}")
                for dc in range(DCHUNKS):
                    nc.tensor.transpose(
                        psbC[:, dc * P : (dc + 1) * P],
                        XC[:, 0, dc * P : (dc + 1) * P],
                        ident,
                    )
                nc.any.tensor_copy(
                    Tc[:, :, lc, :],
                    psbC[:, :768].rearrange("p (dc f) -> p dc f", f=P),
                )
                for dc in range(DCHUNKS):
                    off = (dc * 2 + lc) * P
                    nc.tensor.matmul(
                        ps[:, lc],
                        T[:, off : off + P],
                        T[:, 1536 + dc * 2 * P : 1536 + (dc + 1) * 2 * P],
                        start=(dc == 0),
                        stop=(dc == DCHUNKS - 1),
                    )
            nc.vector.reduce_max(
                RM[:, 2 * LAST : 2 * LAST + 2], ps, axis=mybir.AxisListType.X
            )

            # second half of out[pair] = sum over l (sum over 128 partitions)
            nc.tensor.matmul(
                fin[:, PAIRS_PER_CORE :],
                ones,
                RM[:, PAIRS_PER_CORE :],
                start=True,
                stop=True,
            )
            fsb = cpool.tile([1, 2 * PAIRS_PER_CORE], f32)
            nc.vector.tensor_copy(fsb, fin)
            osb = cpool.tile([1, PAIRS_PER_CORE], f32)
            fsb2 = fsb.rearrange("p (n two) -> p n two", two=2)
            nc.vector.tensor_tensor(
                osb, fsb2[:, :, 0], fsb2[:, :, 1], op=mybir.AluOpType.add
            )
            nc.sync.dma_start(out, osb)

    nc.compile()
    return nc


def _get_nc():
    if "nc" not in _cache:
        _cache["nc"] = _build()
    return _cache["nc"]


def run(context, trace=False, tmpdir=None):
    from concourse import bass_utils

    nc = _get_nc()
    context = np.ascontiguousarray(np.asarray(context, dtype=np.float32))
    assert context.shape == (B, TOPK, 2, L, D), context.shape
    in_maps = [{"x": context[c]} for c in range(N_CORES)]
    res = bass_utils.run_bass_kernel_spmd(
        nc, in_maps, core_ids=list(range(N_CORES)), trace=trace, tmpdir=tmpdir
    )
    out = np.concatenate(
        [res.results[c]["out"].reshape(1, PAIRS_PER_CORE) for c in range(N_CORES)],
        axis=0,
    ).astype(np.float32)
    return out, res


def kernel(context):
    out, _ = run(context, trace=False)
    return out


# revision 22
# speedup vs baseline: 1.1590x; 1.1590x over previous
"""Trainium2 Bass kernel for nn_MlpwithSOMModule (sum-of-max hard-attention score).

Math identity used: the reference computes
    sim  = ctx @ ent^T            # [L, M] per (b, k)
    idx  = argmax_m sim
    out  = sum_l dot(ctx_l, ent_idx[l]) = sum_l sim[l, idx[l]] = sum_l max_m sim[l, m]
so no gather/argmax is needed on device - just matmul, row-max, and a sum.

Sharding: B = 8 == n_cores, so core c processes context[c] = [64, 2, 256, 768]
(its 64 (b,k) pairs are exactly batch b == c). No cross-core communication.

Per-core pipeline (Tile framework):
  1. SWDGE DMA loads ONE pair per dma_start (63 loads + a 3-piece final pair),
     casting fp32 -> bf16 in the DMA. Tokens map to partitions as l = 2p + r
     so each partition reads 6KB contiguous bursts. Per-pair loads give the
     finest possible semaphore granularity: the PE never waits on more than
     one pair's worth of HBM bytes, so its lag behind the DMA stream stays
     bounded at ~1 pair instead of growing to a multi-pair backlog that gets
     exposed as a tail after the last byte lands. xpool is 24 buffers deep so
     dma_start issue runs far ahead of PE consumption and the SDMA queue
     never starves (issue is gated by buffer recycling; shallow buffering
     couples DMA issue to PE progress and collapses the sustained HBM rate
     from ~425 GB/s to ~330).
  2. PE transposes 24 [128,128] bf16 blocks per pair (ctx and ent to
     [d, token]) into bf16 PSUM slabs; ACT/DVE copy 1024-wide slabs to SBUF.
     The PE stream staggers one pair: [T(p+1), MM(p)] so MM(p) never waits
     on its own slab copies.
  3. 12 accumulating bf16 matmuls per pair: S[l, m] (contraction d = 6 x 128);
     DVE reduce_max over m -> row maxes in a [128, 128] SBUF tile (RM).
  4. Final sum over l = ones^T @ RM, split in two halves: the first half is
     emitted right after pair 31 so only the second 64-column matmul plus a
     copy/fold/store chain depends on the last pair.
"""

import sys

for _p in ("/opt/trn_rl_repo", "/root/.axon_site/_ro/trn_rl_repo"):
    if _p not in sys.path:
        sys.path.insert(0, _p)

import numpy as np

B, TOPK, L, D = 8, 64, 256, 768
N_CORES = 8
PAIRS_PER_CORE = 64  # == TOPK; one batch index per core
P = 128
DCHUNKS = D // P  # 6
LCHUNKS = L // P  # 2

_cache = {}


def _build():
    import concourse.bass as bass
    import concourse.mybir as mybir
    from concourse import bacc
    from concourse.tile import TileContext
    from concourse.masks import make_identity

    nc = bacc.Bacc(
        "TRN2",
        target_bir_lowering=False,
        debug=False,
        num_devices=N_CORES,
    )

    x = nc.dram_tensor(
        "x", [PAIRS_PER_CORE, 2, L, D], mybir.dt.float32, kind="ExternalInput"
    ).ap()
    out = nc.dram_tensor(
        "out", [1, PAIRS_PER_CORE], mybir.dt.float32, kind="ExternalOutput"
    ).ap()

    bf16 = mybir.dt.bfloat16
    f32 = mybir.dt.float32

    # DRAM view: pair pr, partition p, chunk c=(s,r), d -- token l = 2p + r,
    # so each partition reads 2 consecutive rows (6KB contiguous DMA bursts).
    # The token permutation is harmless: out = sum_l max_m is invariant to the
    # order of l and m, and the permutation is identical across all d-chunks.
    xv = x.rearrange("pr s (p two) d -> pr p s two d", p=P, two=2)

    with TileContext(nc) as tc:
        with (
            tc.tile_pool(name="const", bufs=1) as cpool,
            tc.tile_pool(name="xload", bufs=24) as xpool,
            tc.tile_pool(name="xfinal", bufs=3) as xfpool,
            tc.tile_pool(name="tpose", bufs=3) as tpool,
            tc.tile_pool(name="ppose", bufs=4, space="PSUM") as ppool,
            tc.tile_pool(name="pmm", bufs=3, space="PSUM") as mpool,
            tc.tile_pool(name="pfin", bufs=1, space="PSUM") as fpool,
        ):
            ident = cpool.tile([P, P], bf16)
            ones = cpool.tile([P, 1], f32)
            # row maxes: column 2*pair+lc holds max_m S[l, m] for l-chunk lc
            RM = cpool.tile([P, 2 * PAIRS_PER_CORE], f32)
            fin = fpool.tile([1, 2 * PAIRS_PER_CORE], f32)

            def emit_mm(pair, T):
                ps = mpool.tile([P, LCHUNKS, 2 * P], f32)
                for lc in range(LCHUNKS):
                    for dc in range(DCHUNKS):
                        off = (dc * 2 + lc) * P
                        nc.tensor.matmul(
                            ps[:, lc],
                            T[:, off : off + P],  # ctxT block [d, l-chunk]
                            T[:, 1536 + dc * 2 * P : 1536 + (dc + 1) * 2 * P],
                            start=(dc == 0),
                            stop=(dc == DCHUNKS - 1),
                        )
                nc.vector.reduce_max(
                    RM[:, 2 * pair : 2 * pair + 2], ps, axis=mybir.AxisListType.X
                )
                if pair == PAIRS_PER_CORE // 2 - 1:
                    # first half of the final sum over l: only the second half
                    # remains on the critical path after the last pair
                    nc.tensor.matmul(
                        fin[:, : PAIRS_PER_CORE],
                        ones,
                        RM[:, : PAIRS_PER_CORE],
                        start=True,
                        stop=True,
                    )

            # Load plan: one pair per dma_start. Finest semaphore
            # granularity (PE backlog stays ~1 pair when the stream ends) and
            # the smoothest HBM demand curve; with the 24-deep xpool the
            # descgen issue runs far ahead of PE consumption. The final pair
            # is loaded in 3 pieces below.
            sizes = [1] * (PAIRS_PER_CORE - 1)
            assert sum(sizes) == PAIRS_PER_CORE - 1
            loads = []
            s0 = 0
            for n in sizes:
                loads.append((s0, n))
                s0 += n

            # Pair 0 via HWDGE (sync engine): RTL descriptor generation has
            # ~0.6us first-byte latency and runs concurrently with the Q7/
            # SWDGE warmup (~3us) for pair 1, so bytes start flowing ~2.5us
            # sooner. HWDGE cannot cast, so load fp32 and fold to bf16 on DVE.
            X32 = cpool.tile([P, 4, D], f32)
            nc.sync.dma_start(
                X32, xv[0:1].rearrange("n p s two d -> p (n s) (two d)")
            )
            prev = None
            for li, (p0, n) in enumerate(loads):
                Xfull = xpool.tile([P, 4, D], bf16, tag="X", name="X")
                X = Xfull[:, : 4 * n, :]
                if li == 0:
                    make_identity(nc, ident)
                    nc.gpsimd.memset(ones, 1.0)
                    nc.vector.tensor_copy(X, X32)
                else:
                    # fp32 -> bf16 cast in DMA (SWDGE)
                    nc.gpsimd.dma_start(
                        X, xv[p0 : p0 + n].rearrange("n p s two d -> p (n s) (two d)")
                    )
                for q in range(n):
                    if prev is not None:
                        emit_mm(*prev)
                    # T: ctxT at [0, 1536), entT at [1536, 3072); block
                    # (t, dc, lc) lives at free offset 128*(t*12 + dc*2 + lc)
                    T = tpool.tile([P, 2 * 1536], bf16)
                    for jj in range(3):
                        psb = ppool.tile([P, 1024], bf16)
                        for slot in range(8):
                            j = jj * 8 + slot
                            t, rem = divmod(j, 12)
                            dc, lc = divmod(rem, 2)
                            c = q * 4 + t * 2 + lc
                            nc.tensor.transpose(
                                psb[:, slot * P : (slot + 1) * P],
                                X[:, c, dc * P : (dc + 1) * P],
                                ident,
                            )
                        # deterministic 2:1 ACT/DVE split keeps DVE light (it
                        # also runs the per-pair reduce_max) and ACT under the
                        # per-pair load cadence
                        dst = T[:, jj * 1024 : (jj + 1) * 1024]
                        if jj == 1:
                            nc.vector.tensor_copy(dst, psb)
                        else:
                            nc.scalar.copy(dst, psb)
                    prev = (p0 + q, T)

            # Final pair (63): loaded in 3 pieces (ent, ctx l-chunk 0, ctx
            # l-chunk 1) so the only compute gated on the very last HBM bytes
            # is 6 transposes + 6 matmuls instead of a full pair.
            LAST = PAIRS_PER_CORE - 1
            xl = xv[LAST]
            XE = xfpool.tile([P, 2, D], bf16, name="XE")
            nc.gpsimd.dma_start(XE, xl[:, 1])
            XC0 = xfpool.tile([P, 1, D], bf16, name="XC0")
            nc.gpsimd.dma_start(XC0, xl[:, 0, 0:1, :])
            XC1 = xfpool.tile([P, 1, D], bf16, name="XC1")
            nc.gpsimd.dma_start(XC1, xl[:, 0, 1:2, :])

            emit_mm(*prev)

            T = tpool.tile([P, 2 * 1536], bf16, tag="T", name="Tlast")
            Tc = T[:, :1536].rearrange("p (dc two f) -> p dc two f", two=2, f=P)
            # ent blocks j=12..23 -> T[1536:3072]
            psbA = ppool.tile([P, 1024], bf16, tag="psb", name="psbA")
            for k in range(8):
                dc, lc = divmod(k, 2)
                nc.tensor.transpose(
                    psbA[:, k * P : (k + 1) * P], XE[:, lc, dc * P : (dc + 1) * P], ident
                )
            nc.any.tensor_copy(T[:, 1536:2560], psbA)
            psbB = ppool.tile([P, 1024], bf16, tag="psb", name="psbB")
            for k in range(4):
                dc, lc = divmod(k + 8, 2)
                nc.tensor.transpose(
                    psbB[:, k * P : (k + 1) * P], XE[:, lc, dc * P : (dc + 1) * P], ident
                )
            nc.any.tensor_copy(T[:, 2560:3072], psbB[:, :512])
            ps = mpool.tile([P, LCHUNKS, 2 * P], f32, tag="ps", name="ps_last")
            for lc, XC in ((0, XC0), (1, XC1)):
                psbC = ppool.tile([P, 1024], bf16, tag="psb", name=f"psbC{lc}")
                for dc in range(DCHUNKS):
                    nc.tensor.transpose(
                        psbC[:, dc * P : (dc + 1) * P],
                        XC[:, 0, dc * P : (dc + 1) * P],
                        ident,
                    )
                nc.any.tensor_copy(
                    Tc[:, :, lc, :],
                    psbC[:, :768].rearrange("p (dc f) -> p dc f", f=P),
                )
                for dc in range(DCHUNKS):
                    off = (dc * 2 + lc) * P
                    nc.tensor.matmul(
                        ps[:, lc],
                        T[:, off : off + P],
                        T[:, 1536 + dc * 2 * P : 1536 + (dc + 1) * 2 * P],
                        start=(dc == 0),
                        stop=(dc == DCHUNKS - 1),
                    )
            nc.vector.reduce_max(
                RM[:, 2 * LAST : 2 * LAST + 2], ps, axis=mybir.AxisListType.X
            )

            # second half of out[pair] = sum over l (sum over 128 partitions)
            nc.tensor.matmul(
                fin[:, PAIRS_PER_CORE :],
                ones,
                RM[:, PAIRS_PER_CORE :],
                start=True,
                stop=True,
            )
            fsb = cpool.tile([1, 2 * PAIRS_PER_CORE], f32)
            nc.vector.tensor_copy(fsb, fin)
            osb = cpool.tile([1, PAIRS_PER_CORE], f32)
            fsb2 = fsb.rearrange("p (n two) -> p n two", two=2)
            nc.vector.tensor_tensor(
                osb, fsb2[:, :, 0], fsb2[:, :, 1], op=mybir.AluOpType.add
            )
            nc.sync.dma_start(out, osb)

    nc.compile()
    return nc


def _get_nc():
    if "nc" not in _cache:
        _cache["nc"] = _build()
    return _cache["nc"]


def run(context, trace=False, tmpdir=None):
    from concourse import bass_utils

    nc = _get_nc()
    context = np.ascontiguousarray(np.asarray(context, dtype=np.float32))
    assert context.shape == (B, TOPK, 2, L, D), context.shape
    in_maps = [{"x": context[c]} for c in range(N_CORES)]
    res = bass_utils.run_bass_kernel_spmd(
        nc, in_maps, core_ids=list(range(N_CORES)), trace=trace, tmpdir=tmpdir
    )
    out = np.concatenate(
        [res.results[c]["out"].reshape(1, PAIRS_PER_CORE) for c in range(N_CORES)],
        axis=0,
    ).astype(np.float32)
    return out, res


def kernel(context):
    out, _ = run(context, trace=False)
    return out
